# revision 12
# baseline (speedup 1.0000x reference)
"""Causal multi-head attention (GPT-2 style Conv1D attention) on 8 Trainium2 cores.

Strategy: tensor-parallel over heads. Each core owns 2 of the 16 heads (both
batches). Everything on-chip is computed in transposed layout:

  - hidden^T tiles [c, t] built by PE transpose
  - q^T/k^T/v^T = w_slice.T @ hidden^T   (PSUM->SBUF copy adds b_attn bias)
  - scores^T[tk, tq] = k^T.T @ q^T  directly (K=64 contraction)
  - causal mask added in PSUM via identity @ mask_pattern matmul
  - exp on ScalarE: exp(0.125*scores + attention_mask[tk]) -> unnormalized A^T
  - attn@V with a fused ones column:  [v | 1].T @ A^T gives out^T rows 0:63
    and the softmax denominator s[tq] in row 64
  - A^T tiles are DMAd out unnormalized; host multiplies by r=1/s and
    transposes (fully-masked tiles are never written; outputs are pre-zeroed)
  - out^T rows are normalized on-device (r broadcast via K=1 ones matmul),
    then c_proj partial = out^T.T @ w_proj_rows; partials summed on host.

Matmuls run as float32r (TF32-like, ~1.6e-4 rel err, 4x faster than fp32).
"""

import numpy as np

import concourse.bass as bass
import concourse.mybir as mybir
import concourse.tile as tile
from concourse import bacc
from concourse.bass_utils import run_bass_kernel_spmd
from concourse.masks import make_identity

F32 = mybir.dt.float32
F32R = mybir.dt.float32r
Exp = mybir.ActivationFunctionType.Exp
Ident = mybir.ActivationFunctionType.Identity

B, S, D, H = 2, 2048, 1024, 16
HD = D // H            # 64
NCORES = 8
HL = H // NCORES       # 2 heads per core
T = B * S              # 4096 tokens across both batches
P = 128
NTB = T // 512         # 8 token blocks (phase 1)
NKT = S // P           # 16 tk tiles per sequence
NJ = S // 512          # 4 tq blocks per sequence
MASK_VAL = -80000.0    # *0.125 -> -10000 (reference's masked bias)


def build_program():
    nc = bacc.Bacc("TRN2", target_bir_lowering=False, debug=False)

    hid_d = nc.dram_tensor("hidden", [T, D], F32, kind="ExternalInput")
    wqkv_d = nc.dram_tensor("wqkv", [D, 3 * P], F32R, kind="ExternalInput")
    bqkv_d = nc.dram_tensor("bqkv", [3 * P], F32, kind="ExternalInput")
    am_d = nc.dram_tensor("amask", [P, B * NKT], F32, kind="ExternalInput")
    mpat_d = nc.dram_tensor("maskpat", [P, 4, 512], F32R, kind="ExternalInput")
    wproj_d = nc.dram_tensor("wproj", [P, D], F32R, kind="ExternalInput")

    attn_d = nc.dram_tensor("attn_t", [B, HL, S, S], F32, kind="ExternalOutput")
    r_d = nc.dram_tensor("r_out", [B, HL, NJ, 1, 512], F32, kind="ExternalOutput")
    cproj_d = nc.dram_tensor("cproj", [T, D], F32, kind="ExternalOutput")

    with tile.TileContext(nc) as tc:
        with (
            tc.tile_pool(name="const", bufs=1) as cst,
            tc.tile_pool(name="hid", bufs=2) as hid_pool,
            tc.tile_pool(name="hidT", bufs=2) as hidT_pool,
            tc.tile_pool(name="qkvT", bufs=1) as qkv_pool,
            tc.tile_pool(name="vaug", bufs=2) as vaug_pool,
            tc.tile_pool(name="attn", bufs=7) as attn_pool,
            tc.tile_pool(name="small", bufs=2) as small_pool,
            tc.tile_pool(name="cstage", bufs=2) as cstage_pool,
            tc.tile_pool(name="psA", bufs=3, space="PSUM") as psA,   # accumulators
            tc.tile_pool(name="psB", bufs=3, space="PSUM") as psB,   # transposes / scores
            tc.tile_pool(name="psC", bufs=2, space="PSUM") as psC,   # r broadcast
        ):
            # ---- constants ----
            ident = cst.tile([P, P], F32, tag="ident")
            make_identity(nc, ident[:])
            identr = cst.tile([P, P], F32R, tag="identr")
            nc.vector.tensor_copy(identr[:], ident[:])
            onesf = cst.tile([1, P], F32, tag="onesf")
            nc.gpsimd.memset(onesf[:], 1.0)
            ones1 = cst.tile([1, P], F32R, tag="ones1")
            nc.vector.tensor_copy(ones1[:], onesf[:])
            ones_col = cst.tile([P, 1], F32, tag="ones_col")
            nc.gpsimd.memset(ones_col[:], 1.0)
            mpat = cst.tile([P, 4, 512], F32R, tag="mpat")
            nc.sync.dma_start(mpat[:], mpat_d.ap())
            am = cst.tile([P, B * NKT], F32, tag="am")
            nc.sync.dma_start(am[:], am_d.ap())
            bqkv = cst.tile([P, 3], F32, tag="bqkv")
            nc.sync.dma_start(bqkv[:], bqkv_d.ap().rearrange("(x p) -> p x", p=P))
            w_sb = cst.tile([P, D // P, 3 * P], F32R, tag="w_sb")
            nc.sync.dma_start(w_sb[:], wqkv_d.ap().rearrange("(o p) m -> p o m", p=P))
            wproj = cst.tile([P, D], F32R, tag="wproj")
            nc.sync.dma_start(wproj[:], wproj_d.ap())

            # persistent transposed q/k/v: [d' (2 heads x 64), t]
            qT = qkv_pool.tile([P, T], F32R, tag="qT")
            kT = qkv_pool.tile([P, T], F32R, tag="kT")
            vT = qkv_pool.tile([P, T], F32R, tag="vT")
            qkvT = [qT, kT, vT]
            # normalized attention output (transposed), per batch
            aoT = [
                qkv_pool.tile([P, S], F32R, tag=f"aoT{b}", name=f"aoT{b}")
                for b in range(B)
            ]

            # ---- phase 1: hidden^T and q/k/v^T ----
            for tb in range(NTB):
                hid_nat = hid_pool.tile([P, 4, D], F32, tag="hid_nat")
                nc.sync.dma_start(
                    hid_nat[:],
                    hid_d.ap()[tb * 512 : (tb + 1) * 512, :].rearrange(
                        "(n p) c -> p n c", p=P
                    ),
                )
                hidT = hidT_pool.tile([P, D // P, 512], F32R, tag="hidT")
                for c in range(D // P):
                    ptr = psB.tile([P, 512], F32, tag="ptr")
                    for n in range(4):
                        nc.tensor.transpose(
                            ptr[:, n * P : (n + 1) * P],
                            hid_nat[:, n, c * P : (c + 1) * P],
                            ident[:],
                        )
                    nc.vector.tensor_copy(hidT[:, c, :], ptr[:])
                for x in range(3):
                    pq = psA.tile([P, 512], F32, tag="acc")
                    for c in range(D // P):
                        nc.tensor.matmul(
                            pq[:],
                            w_sb[:, c, x * P : (x + 1) * P],
                            hidT[:, c, :],
                            start=(c == 0),
                            stop=(c == D // P - 1),
                        )
                    nc.scalar.activation(
                        qkvT[x][:, tb * 512 : (tb + 1) * 512],
                        pq[:],
                        Ident,
                        bias=bqkv[:, x : x + 1],
                        scale=1.0,
                    )

            # ---- phase 2: attention per (batch, local head) ----
            for b in range(B):
                for hl in range(HL):
                    hb = hl * HD           # partition base of this head (0 or 64)
                    t0 = b * S             # token offset of this batch
                    kT_h = kT[hb : hb + HD, t0 : t0 + S]
                    qT_h = qT[hb : hb + HD, t0 : t0 + S]
                    vT_h = vT[hb : hb + HD, t0 : t0 + S]

                    # build v_aug [tk, 16, 65]: columns 0:64 = v, column 64 = 1.0
                    vaug = vaug_pool.tile([P, NKT, HD + 1], F32R, tag="vaug")
                    nc.vector.tensor_copy(
                        vaug[:, :, HD : HD + 1],
                        ones_col[:, None, :].to_broadcast((P, NKT, 1)),
                    )
                    for g in range(2):
                        pvt = psC.tile([P, 512], F32R, tag="rb")
                        for u in range(8):
                            i = g * 8 + u
                            nc.tensor.transpose(
                                pvt[:, u * HD : (u + 1) * HD],
                                vT_h[:, i * P : (i + 1) * P],
                                identr[hb : hb + HD, hb : hb + HD],
                            )
                        nc.vector.tensor_copy(
                            vaug[:, g * 8 : (g + 1) * 8, 0:HD],
                            pvt[:].rearrange("p (u d) -> p u d", d=HD),
                        )

                    for j in range(NJ):
                        nI = min(4 * j + 4, NKT)
                        pav = psA.tile([P, 512], F32, tag="acc")
                        for i in range(nI):
                            ps = psB.tile([P, 512], F32, tag="ptr")
                            d = i - 4 * j
                            diag = 0 <= d <= 3
                            nc.tensor.matmul(
                                ps[:],
                                kT_h[:, i * P : (i + 1) * P],
                                qT_h[:, j * 512 : (j + 1) * 512],
                                start=True,
                                stop=not diag,
                            )
                            if diag:
                                nc.tensor.matmul(
                                    ps[:], identr[:], mpat[:, d, :],
                                    start=False, stop=True,
                                )
                            a_i = attn_pool.tile([P, 512], F32R, tag="a")
                            nc.scalar.activation(
                                a_i[:], ps[:], Exp,
                                bias=am[:, b * NKT + i : b * NKT + i + 1],
                                scale=0.125,
                            )
                            nc.sync.dma_start(
                                attn_d.ap()[
                                    b, hl,
                                    i * P : (i + 1) * P,
                                    j * 512 : (j + 1) * 512,
                                ],
                                a_i[:].bitcast(F32),
                            )
                            nc.tensor.matmul(
                                pav[: HD + 1, :],
                                vaug[:, i, :],
                                a_i[:],
                                start=(i == 0),
                                stop=(i == nI - 1),
                            )
                        r_sb = small_pool.tile([1, 512], F32, tag="r")
                        nc.vector.reciprocal(r_sb[:], pav[HD : HD + 1, :])
                        nc.sync.dma_start(r_d.ap()[b, hl, j], r_sb[:])
                        rr_sb = small_pool.tile([1, 512], F32R, tag="rr")
                        nc.vector.tensor_copy(rr_sb[:], r_sb[:])
                        prb = psC.tile([P, 512], F32, tag="rb")
                        nc.tensor.matmul(
                            prb[:HD, :], ones1[:, :HD], rr_sb[:],
                            start=True, stop=True,
                        )
                        rb_sb = small_pool.tile([HD, 512], F32, tag="rbs")
                        nc.vector.tensor_copy(rb_sb[:], prb[:HD, :])
                        nc.vector.tensor_tensor(
                            aoT[b][hb : hb + HD, j * 512 : (j + 1) * 512],
                            pav[:HD, :],
                            rb_sb[:],
                            mybir.AluOpType.mult,
                        )

            # ---- phase 3: c_proj partial [T, D] ----
            for b in range(B):
                for tt in range(S // P):
                    for nb in range(2):
                        pc = psA.tile([P, 512], F32, tag="acc")
                        nc.tensor.matmul(
                            pc[:],
                            aoT[b][:, tt * P : (tt + 1) * P],
                            wproj[:, nb * 512 : (nb + 1) * 512],
                            start=True,
                            stop=True,
                        )
                        cstage = cstage_pool.tile([P, 512], F32, tag="cstage")
                        nc.vector.tensor_copy(cstage[:], pc[:])
                        nc.sync.dma_start(
                            cproj_d.ap()[
                                b * S + tt * P : b * S + (tt + 1) * P,
                                nb * 512 : (nb + 1) * 512,
                            ],
                            cstage[:],
                        )

    nc.compile()
    return nc


_NC_CACHE = None


def _get_program():
    global _NC_CACHE
    if _NC_CACHE is None:
        _NC_CACHE = build_program()
    return _NC_CACHE


def _shard_inputs(hidden_states, attention_mask, w_attn, b_attn, w_proj, b_proj):
    hid = np.ascontiguousarray(hidden_states.reshape(T, D), dtype=np.float32)
    am2 = np.asarray(attention_mask, dtype=np.float32).reshape(B, S)
    # per-partition layout: am[p, b*16+i] = mask[b, 128*i + p]
    am_sb = np.ascontiguousarray(
        am2.reshape(B, NKT, P).transpose(2, 0, 1).reshape(P, B * NKT)
    )
    # causal mask patterns for diagonal tiles, offset d in {0,1,2,3}:
    # masked (tk > tq) where f < p + 128*d
    pp, ff = np.meshgrid(np.arange(P), np.arange(512), indexing="ij")
    mpat = np.stack(
        [np.where(ff < pp + 128 * d, MASK_VAL, 0.0) for d in range(4)], axis=1
    ).astype(np.float32)
    mpat = np.ascontiguousarray(mpat)

    w_attn = np.asarray(w_attn, dtype=np.float32)
    b_attn = np.asarray(b_attn, dtype=np.float32)
    w_proj = np.asarray(w_proj, dtype=np.float32)

    in_maps = []
    for r in range(NCORES):
        c0 = r * P  # first column of this core's 2-head block
        wqkv = np.concatenate(
            [w_attn[:, c0 : c0 + P],
             w_attn[:, D + c0 : D + c0 + P],
             w_attn[:, 2 * D + c0 : 2 * D + c0 + P]],
            axis=1,
        )
        bqkv = np.concatenate(
            [b_attn[c0 : c0 + P], b_attn[D + c0 : D + c0 + P],
             b_attn[2 * D + c0 : 2 * D + c0 + P]]
        )
        in_maps.append(
            {
                "hidden": hid,
                "wqkv": np.ascontiguousarray(wqkv),
                "bqkv": np.ascontiguousarray(bqkv),
                "amask": am_sb,
                "maskpat": mpat,
                "wproj": np.ascontiguousarray(w_proj[c0 : c0 + P, :]),
            }
        )
    return in_maps


def kernel(hidden_states, attention_mask, w_attn, b_attn, w_proj, b_proj, _results=None):
    nc = _get_program()
    in_maps = _shard_inputs(
        hidden_states, attention_mask, w_attn, b_attn, w_proj, b_proj
    )
    if _results is None:
        _results = run_bass_kernel_spmd(nc, in_maps, list(range(NCORES))).results

    attn_w = np.empty((B, H, S, S), dtype=np.float32)
    cproj = np.zeros((T, D), dtype=np.float32)
    for r in range(NCORES):
        res = _results[r]
        a_t = res["attn_t"]            # [B, HL, tk, tq], unnormalized
        r_v = res["r_out"].reshape(B, HL, S)
        for b in range(B):
            for hl in range(HL):
                attn_w[b, HL * r + hl] = (a_t[b, hl] * r_v[b, hl][None, :]).T
        cproj += res["cproj"]

    attn_out = (cproj + np.asarray(b_proj, dtype=np.float32)[None, :]).reshape(B, S, D)
    return attn_out, attn_w


# revision 15
# speedup vs baseline: 53.1383x; 53.1383x over previous
"""Causal multi-head attention (GPT-2 style Conv1D attention) on 8 Trainium2 cores.

Strategy: tensor-parallel over heads. Each core owns 2 of the 16 heads (both
batches). Everything on-chip is computed in transposed layout:

  - hidden^T tiles [c, t] built by PE transpose
  - q^T/k^T/v^T = w_slice.T @ hidden^T   (PSUM->SBUF copy adds b_attn bias)
  - scores^T[tk, tq] = k^T.T @ q^T  directly (K=64 contraction)
  - causal mask added in PSUM via identity @ mask_pattern matmul
  - exp on ScalarE: exp(0.125*scores + attention_mask[tk]) -> unnormalized A^T
  - attn@V with a fused ones column:  [v | 1].T @ A^T gives out^T rows 0:63
    and the softmax denominator s[tq] in row 64
  - A^T tiles are DMAd out unnormalized in 1MB groups; host multiplies by
    r=1/s and transposes (fully-masked tiles never written; outputs pre-zeroed)
  - out^T rows are normalized on-device (r broadcast via K=1 ones matmul),
    then c_proj partial = out^T.T @ w_proj_rows; partials summed on host.

The whole pipeline runs per-batch (QKV -> attention -> c_proj) so DMA/PE/ACT
overlap across phases. Matmuls run as float32r (TF32-like, ~1.6e-4 rel err,
4x faster than fp32).
"""

import numpy as np

import concourse.bass as bass
import concourse.mybir as mybir
import concourse.tile as tile
from concourse import bacc
from concourse.bass_utils import run_bass_kernel_spmd
from concourse.masks import make_identity

F32 = mybir.dt.float32
F32R = mybir.dt.float32r
Exp = mybir.ActivationFunctionType.Exp
Ident = mybir.ActivationFunctionType.Identity

B, S, D, H = 2, 2048, 1024, 16
HD = D // H            # 64
NCORES = 8
HL = H // NCORES       # 2 heads per core
T = B * S              # 4096 tokens across both batches
P = 128
NKT = S // P           # 16 tk tiles per sequence
NJ = S // 512          # 4 tq blocks per sequence
MASK_VAL = -80000.0    # *0.125 -> -10000 (reference's masked bias)


def build_program(repeat=1):
    nc = bacc.Bacc("TRN2", target_bir_lowering=False, debug=False)

    hid_d = nc.dram_tensor("hidden", [T, D], F32, kind="ExternalInput")
    wqkv_d = nc.dram_tensor("wqkv", [D, 3 * P], F32R, kind="ExternalInput")
    bqkv_d = nc.dram_tensor("bqkv", [3 * P], F32, kind="ExternalInput")
    am_d = nc.dram_tensor("amask", [P, B * NKT], F32, kind="ExternalInput")
    mpat_d = nc.dram_tensor("maskpat", [P, 4, 512], F32R, kind="ExternalInput")
    wproj_d = nc.dram_tensor("wproj", [P, D], F32R, kind="ExternalInput")

    attn_d = nc.dram_tensor("attn_t", [B, HL, S, S], F32, kind="ExternalOutput")
    r_d = nc.dram_tensor("r_out", [B, HL, NJ, 1, 512], F32, kind="ExternalOutput")
    cproj_d = nc.dram_tensor("cproj", [T, D], F32, kind="ExternalOutput")

    with tile.TileContext(nc) as tc:
        with (
            tc.tile_pool(name="const", bufs=1) as cst,
            tc.tile_pool(name="hid", bufs=2) as hid_pool,
            tc.tile_pool(name="hidT", bufs=2) as hidT_pool,
            tc.tile_pool(name="qkvT", bufs=2) as qkv_pool,
            tc.tile_pool(name="vaug", bufs=2) as vaug_pool,
            tc.tile_pool(name="attn", bufs=3) as attn_pool,
            tc.tile_pool(name="small", bufs=2) as small_pool,
            tc.tile_pool(name="cstage", bufs=2) as cstage_pool,
            tc.tile_pool(name="psA", bufs=3, space="PSUM") as psA,   # accumulators
            tc.tile_pool(name="psB", bufs=3, space="PSUM") as psB,   # transposes / scores
            tc.tile_pool(name="psC", bufs=2, space="PSUM") as psC,   # r broadcast
        ):
            # ---- constants ----
            ident = cst.tile([P, P], F32, tag="ident")
            make_identity(nc, ident[:])
            identr = cst.tile([P, P], F32R, tag="identr")
            nc.vector.tensor_copy(identr[:], ident[:])
            onesf = cst.tile([1, P], F32, tag="onesf")
            nc.gpsimd.memset(onesf[:], 1.0)
            ones1 = cst.tile([1, P], F32R, tag="ones1")
            nc.vector.tensor_copy(ones1[:], onesf[:])
            ones_col = cst.tile([P, 1], F32, tag="ones_col")
            nc.gpsimd.memset(ones_col[:], 1.0)
            mpat = cst.tile([P, 4, 512], F32R, tag="mpat")
            nc.sync.dma_start(mpat[:], mpat_d.ap())
            am = cst.tile([P, B * NKT], F32, tag="am")
            nc.sync.dma_start(am[:], am_d.ap())
            bqkv = cst.tile([P, 3], F32, tag="bqkv")
            nc.sync.dma_start(bqkv[:], bqkv_d.ap().rearrange("(x p) -> p x", p=P))
            w_sb = cst.tile([P, D // P, 3 * P], F32R, tag="w_sb")
            nc.sync.dma_start(w_sb[:], wqkv_d.ap().rearrange("(o p) m -> p o m", p=P))
            wproj = cst.tile([P, D], F32R, tag="wproj")
            nc.sync.dma_start(wproj[:], wproj_d.ap())

            def emit_batch(b):
                t0 = b * S
                # ---- QKV^T for this batch: q/k/v^T [d' (2 heads x 64), t] ----
                qT = qkv_pool.tile([P, S], F32R, tag="qT", name=f"qT{b}")
                kT = qkv_pool.tile([P, S], F32R, tag="kT", name=f"kT{b}")
                vT = qkv_pool.tile([P, S], F32R, tag="vT", name=f"vT{b}")
                qkvT = [qT, kT, vT]
                aoT = qkv_pool.tile([P, S], F32R, tag="aoT", name=f"aoT{b}")

                for tb in range(S // 512):
                    hid_nat = hid_pool.tile([P, 4, D], F32, tag="hid_nat")
                    nc.sync.dma_start(
                        hid_nat[:],
                        hid_d.ap()[
                            t0 + tb * 512 : t0 + (tb + 1) * 512, :
                        ].rearrange("(n p) c -> p n c", p=P),
                    )
                    hidT = hidT_pool.tile([P, D // P, 512], F32R, tag="hidT")
                    for c in range(D // P):
                        ptr = psB.tile([P, 512], F32, tag="ptr")
                        for n in range(4):
                            nc.tensor.transpose(
                                ptr[:, n * P : (n + 1) * P],
                                hid_nat[:, n, c * P : (c + 1) * P],
                                ident[:],
                            )
                        nc.vector.tensor_copy(hidT[:, c, :], ptr[:])
                    for x in range(3):
                        pq = psA.tile([P, 512], F32, tag="acc")
                        for c in range(D // P):
                            nc.tensor.matmul(
                                pq[:],
                                w_sb[:, c, x * P : (x + 1) * P],
                                hidT[:, c, :],
                                start=(c == 0),
                                stop=(c == D // P - 1),
                            )
                        nc.scalar.activation(
                            qkvT[x][:, tb * 512 : (tb + 1) * 512],
                            pq[:],
                            Ident,
                            bias=bqkv[:, x : x + 1],
                            scale=1.0,
                        )

                # ---- attention per local head ----
                for hl in range(HL):
                    hb = hl * HD           # partition base of this head (0 or 64)
                    kT_h = kT[hb : hb + HD, :]
                    qT_h = qT[hb : hb + HD, :]
                    vT_h = vT[hb : hb + HD, :]

                    # v_aug [tk, 16, 65]: columns 0:64 = v, column 64 = 1.0
                    vaug = vaug_pool.tile([P, NKT, HD + 1], F32R, tag="vaug")
                    nc.vector.tensor_copy(
                        vaug[:, :, HD : HD + 1],
                        ones_col[:, None, :].to_broadcast((P, NKT, 1)),
                    )
                    for g in range(2):
                        pvt = psC.tile([P, 512], F32R, tag="rb")
                        for u in range(8):
                            i = g * 8 + u
                            nc.tensor.transpose(
                                pvt[:, u * HD : (u + 1) * HD],
                                vT_h[:, i * P : (i + 1) * P],
                                identr[hb : hb + HD, hb : hb + HD],
                            )
                        nc.vector.tensor_copy(
                            vaug[:, g * 8 : (g + 1) * 8, 0:HD],
                            pvt[:].rearrange("p (u d) -> p u d", d=HD),
                        )

                    for j in range(NJ):
                        nI = min(4 * j + 4, NKT)
                        pav = psA.tile([P, 512], F32, tag="acc")
                        for i0 in range(0, nI, 4):
                            a4 = attn_pool.tile([P, 4, 512], F32R, tag="a")
                            for u in range(4):
                                i = i0 + u
                                ps = psB.tile([P, 512], F32, tag="ptr")
                                d = i - 4 * j
                                diag = 0 <= d <= 3
                                nc.tensor.matmul(
                                    ps[:],
                                    kT_h[:, i * P : (i + 1) * P],
                                    qT_h[:, j * 512 : (j + 1) * 512],
                                    start=True,
                                    stop=not diag,
                                )
                                if diag:
                                    nc.tensor.matmul(
                                        ps[:], identr[:], mpat[:, d, :],
                                        start=False, stop=True,
                                    )
                                nc.scalar.activation(
                                    a4[:, u, :], ps[:], Exp,
                                    bias=am[:, b * NKT + i : b * NKT + i + 1],
                                    scale=0.125,
                                )
                                nc.tensor.matmul(
                                    pav[: HD + 1, :],
                                    vaug[:, i, :],
                                    a4[:, u, :],
                                    start=(i == 0),
                                    stop=(i == nI - 1),
                                )
                            nc.sync.dma_start(
                                attn_d.ap()[
                                    b, hl,
                                    i0 * P : (i0 + 4) * P,
                                    j * 512 : (j + 1) * 512,
                                ].rearrange("(g p) f -> p g f", p=P),
                                a4[:].bitcast(F32),
                            )
                        r_sb = small_pool.tile([1, 512], F32, tag="r")
                        nc.vector.reciprocal(r_sb[:], pav[HD : HD + 1, :])
                        nc.sync.dma_start(r_d.ap()[b, hl, j], r_sb[:])
                        rr_sb = small_pool.tile([1, 512], F32R, tag="rr")
                        nc.vector.tensor_copy(rr_sb[:], r_sb[:])
                        prb = psC.tile([P, 512], F32, tag="rb")
                        nc.tensor.matmul(
                            prb[:HD, :], ones1[:, :HD], rr_sb[:],
                            start=True, stop=True,
                        )
                        rb_sb = small_pool.tile([HD, 512], F32, tag="rbs")
                        nc.vector.tensor_copy(rb_sb[:], prb[:HD, :])
                        nc.vector.tensor_tensor(
                            aoT[hb : hb + HD, j * 512 : (j + 1) * 512],
                            pav[:HD, :],
                            rb_sb[:],
                            mybir.AluOpType.mult,
                        )

                # ---- c_proj partial for this batch's tokens ----
                for tt in range(S // P):
                    cstage = cstage_pool.tile([P, D], F32, tag="cstage")
                    for nb in range(2):
                        pc = psA.tile([P, 512], F32, tag="acc")
                        nc.tensor.matmul(
                            pc[:],
                            aoT[:, tt * P : (tt + 1) * P],
                            wproj[:, nb * 512 : (nb + 1) * 512],
                            start=True,
                            stop=True,
                        )
                        nc.vector.tensor_copy(
                            cstage[:, nb * 512 : (nb + 1) * 512], pc[:]
                        )
                    nc.sync.dma_start(
                        cproj_d.ap()[t0 + tt * P : t0 + (tt + 1) * P, :],
                        cstage[:],
                    )

            def emit_body():
                for b in range(B):
                    emit_batch(b)

            if repeat == 1:
                emit_body()
            else:
                with tc.For_i(0, repeat, 1):
                    emit_body()

    nc.compile()
    return nc


_NC_CACHE = None


def _get_program():
    global _NC_CACHE
    if _NC_CACHE is None:
        _NC_CACHE = build_program()
    return _NC_CACHE


def _shard_inputs(hidden_states, attention_mask, w_attn, b_attn, w_proj, b_proj):
    hid = np.ascontiguousarray(hidden_states.reshape(T, D), dtype=np.float32)
    am2 = np.asarray(attention_mask, dtype=np.float32).reshape(B, S)
    # per-partition layout: am[p, b*16+i] = mask[b, 128*i + p]
    am_sb = np.ascontiguousarray(
        am2.reshape(B, NKT, P).transpose(2, 0, 1).reshape(P, B * NKT)
    )
    # causal mask patterns for diagonal tiles, offset d in {0,1,2,3}:
    # masked (tk > tq) where f < p + 128*d
    pp, ff = np.meshgrid(np.arange(P), np.arange(512), indexing="ij")
    mpat = np.stack(
        [np.where(ff < pp + 128 * d, MASK_VAL, 0.0) for d in range(4)], axis=1
    ).astype(np.float32)
    mpat = np.ascontiguousarray(mpat)

    w_attn = np.asarray(w_attn, dtype=np.float32)
    b_attn = np.asarray(b_attn, dtype=np.float32)
    w_proj = np.asarray(w_proj, dtype=np.float32)

    in_maps = []
    for r in range(NCORES):
        c0 = r * P  # first column of this core's 2-head block
        wqkv = np.concatenate(
            [w_attn[:, c0 : c0 + P],
             w_attn[:, D + c0 : D + c0 + P],
             w_attn[:, 2 * D + c0 : 2 * D + c0 + P]],
            axis=1,
        )
        bqkv = np.concatenate(
            [b_attn[c0 : c0 + P], b_attn[D + c0 : D + c0 + P],
             b_attn[2 * D + c0 : 2 * D + c0 + P]]
        )
        in_maps.append(
            {
                "hidden": hid,
                "wqkv": np.ascontiguousarray(wqkv),
                "bqkv": np.ascontiguousarray(bqkv),
                "amask": am_sb,
                "maskpat": mpat,
                "wproj": np.ascontiguousarray(w_proj[c0 : c0 + P, :]),
            }
        )
    return in_maps


def kernel(hidden_states, attention_mask, w_attn, b_attn, w_proj, b_proj, _results=None):
    nc = _get_program()
    in_maps = _shard_inputs(
        hidden_states, attention_mask, w_attn, b_attn, w_proj, b_proj
    )
    if _results is None:
        _results = run_bass_kernel_spmd(nc, in_maps, list(range(NCORES))).results

    attn_w = np.empty((B, H, S, S), dtype=np.float32)
    cproj = np.zeros((T, D), dtype=np.float32)
    for r in range(NCORES):
        res = _results[r]
        a_t = res["attn_t"]            # [B, HL, tk, tq], unnormalized
        r_v = res["r_out"].reshape(B, HL, S)
        for b in range(B):
            for hl in range(HL):
                attn_w[b, HL * r + hl] = (a_t[b, hl] * r_v[b, hl][None, :]).T
        cproj += res["cproj"]

    attn_out = (cproj + np.asarray(b_proj, dtype=np.float32)[None, :]).reshape(B, S, D)
    return attn_out, attn_w
